# revision 32
# baseline (speedup 1.0000x reference)
"""Bahdanau attention kernel for Trainium2, 8-core data-parallel.

Problem (B=32, L=1024, H=1024, fp32):
    h     = tanh(q @ W1.T + b1 + v @ W2.T + b2)        # (B, L, H)
    score = h @ Vw.T + vb                              # (B, L, H)
    att   = softmax(score, axis=-1)                    # (B, L, H)
    ctx   = att @ v                                    # (B, L, H)  (bmm over kv dim)
    returns (att, ctx)

Strategy (v3):
  - Data-parallel: 4 batches per core on 8 cores.  Everything on-device runs
    in a TRANSPOSED layout [h, l] so the contraction dim always lands on SBUF
    partitions; host pre-transposes q/value and the weights.
  - The device computes only the three GEMM stages plus tanh/exp; the softmax
    NORMALIZATION runs on the host:  the device outputs the exp numerator
    (f16, also the att output pre-division) and an un-normalized context,
    and the host divides both by S = sum(exp).  This removes the on-device
    softmax-sum matmul, reciprocal, and att-mul, and breaks the
    recip -> mul -> context dependency chain at every step's tail.
  - Stage A/B matmuls run in fp16 (same PE speed as bf16, ~8x less
    quantization error), EXCEPT the first 2 of 8 contraction 128-blocks of
    both stage-A matmuls, which run as one fp8e4m3 DoubleRow matmul per
    operand (256-contraction at the same ~512 cycles, i.e. 2x).  More fp8
    blocks blow the att error budget (3 blocks -> 2.0e-2 in sim); fp8
    anywhere in stage B costs even more error per cycle saved.  The fp8
    pair is scaled as fp8(q/8) x fp8(8*w) -- the scale cancels in the
    product; it keeps the 0.02-std weights out of e4m3's subnormal range
    (which would cost ~15% element error) without pushing q too far in.
  - Context matmul runs entirely in fp8 DoubleRow.  Plain fp8 exp would
    cost ~3.9e-2 rel err (over the 2e-2 budget), so the operand is
    CENTERED:  exp(s) = 1 + expm1(s), where expm1(s) has std ~0.39 vs
    exp's mean ~1.07, so quantizing expm1 into fp8 carries 2.4x less
    absolute error.  The device computes ctx_raw = expm1_8 @ v8 and the
    host adds back the exact rank-1 term 1 @ v = colsum(v) before
    dividing by S.  Total measured on HW: att 1.67e-2, ctx 1.72e-2
    (CPU sim with ml_dtypes predicts HW error to ~2%: sim 1.69e-2).
  - expm1_8 is produced by a DVE tensor_scalar_sub (expT - 1) since the
    scalar engine has no Expm1 and cannot post-subtract; the DVE is
    otherwise nearly idle.
  - Per step (half-batch, 512 l-columns) the PE runs A (128 mm), then the
    PREVIOUS step's context (32 DR mm), then B (64 mm); the gap between
    B(i) and ctx(i) hides the scalar exp + DVE sub latency.
  - Step 0's stage A runs k-block-OUTER with 8 concurrent PSUM accumulation
    groups (all 8 banks), so the first matmul gates on one k-chunk of
    weights+inputs instead of ~3 MiB, and the DMA stream stays ahead of
    the PE for the rest of startup.  PE prewarm matmuls burn the startup
    DMA window so the HAM clock gate (1.2 -> 2.4 GHz after ~3.4us of PE
    activity) releases before the real matmuls start.
  - PSUM accumulation chains are kept short (<= 16); a 512-deep chain was
    observed to hard-crash the exec unit (NRT_EXEC_UNIT_UNRECOVERABLE).
"""

import numpy as np
import ml_dtypes
from contextlib import ExitStack

import concourse.bass as bass
import concourse.mybir as mybir
import concourse.tile as tile
from concourse import bacc, bass_utils

B, L, H = 32, 1024, 1024
NCORES = 8
BLOC = B // NCORES  # batches per core
P = 128             # partitions
LT = 512            # l-tile (moving free dim)
NLB = LT // P       # 128-row blocks per l-tile
NH = H // P         # 128-blocks along h / o / k
NHT = H // LT       # 512-tiles along h (context output)

NA8 = 2             # stage-A contraction 128-blocks (of 8) run in fp8-DR
NHF = NH - NA8      # remaining f16 128-blocks
WS = 8.0            # fp8 pair scale: stores fp8(q/WS) x fp8(WS*w) = q*w;
                    # keeps the 0.02-std weights out of e4m3's subnormal
                    # range without pushing q too far into it

F16 = mybir.dt.float16
F32 = mybir.dt.float32
F8E4 = mybir.dt.float8e4
AFT = mybir.ActivationFunctionType
DR = mybir.MatmulPerfMode.DoubleRow

_PROGRAM_CACHE = {}


def _build_program():
    nc = bacc.Bacc("TRN2", target_bir_lowering=False, debug=False)

    KF = NHF * P  # f16 contraction rows in stage A
    K8 = NA8 * P  # fp8 contraction rows
    qT = nc.dram_tensor("qt_in", [BLOC, KF, L], F16, kind="ExternalInput").ap()
    vT = nc.dram_tensor("vt_in", [BLOC, KF, L], F16, kind="ExternalInput").ap()
    qT8 = nc.dram_tensor("qt8_in", [BLOC, K8, L], F8E4, kind="ExternalInput").ap()
    vT8 = nc.dram_tensor("vt8_in", [BLOC, K8, L], F8E4, kind="ExternalInput").ap()
    vn = nc.dram_tensor("vn_in", [BLOC, L, H], F8E4, kind="ExternalInput").ap()
    w1t = nc.dram_tensor("w1t_in", [KF, H], F16, kind="ExternalInput").ap()
    w2t = nc.dram_tensor("w2t_in", [KF, H], F16, kind="ExternalInput").ap()
    w18 = nc.dram_tensor("w18_in", [K8, H], F8E4, kind="ExternalInput").ap()
    w28 = nc.dram_tensor("w28_in", [K8, H], F8E4, kind="ExternalInput").ap()
    vwt = nc.dram_tensor("vwt_in", [H, H], F16, kind="ExternalInput").ap()
    b12 = nc.dram_tensor("b12_in", [P, NH], F32, kind="ExternalInput").ap()
    vbt = nc.dram_tensor("vbt_in", [P, NH], F32, kind="ExternalInput").ap()

    attT = nc.dram_tensor("att_out", [BLOC, H, L], F16, kind="ExternalOutput").ap()
    ctxo = nc.dram_tensor("ctx_out", [BLOC, L, H], F16, kind="ExternalOutput").ap()

    with tile.TileContext(nc) as tc:
        _kernel_body(tc, qT, vT, qT8, vT8, vn, w1t, w2t, w18, w28, vwt,
                     b12, vbt, attT, ctxo)
    nc.compile()
    return nc


def _kernel_body(tc, qT, vT, qT8, vT8, vn, w1t, w2t, w18, w28, vwt,
                 b12, vbt, attT, ctxo):
    nc = tc.nc
    with ExitStack() as ctx:
        consts = ctx.enter_context(tc.tile_pool(name="consts", bufs=1))
        qpool = ctx.enter_context(tc.tile_pool(name="qpool", bufs=2))
        hpool = ctx.enter_context(tc.tile_pool(name="hpool", bufs=2))
        epool = ctx.enter_context(tc.tile_pool(name="epool", bufs=2))
        vpool = ctx.enter_context(tc.tile_pool(name="vpool", bufs=2))
        cpool = ctx.enter_context(tc.tile_pool(name="cpool", bufs=2))
        psA = ctx.enter_context(tc.tile_pool(name="psA", bufs=2, space="PSUM"))
        psB = ctx.enter_context(tc.tile_pool(name="psB", bufs=2, space="PSUM"))
        psC = ctx.enter_context(tc.tile_pool(name="psC", bufs=4, space="PSUM"))

        # PE prewarm (see module docstring).  The memset must NOT run on
        # GpSimd: its cold-start latency (~9us observed) would gate the
        # prewarm matmuls past the whole startup window.  40 matmuls
        # (~4.3us) deliberately overshoot the first operand chunks' arrival
        # (~9-11.5us wall, ~2us run-to-run variance): the PE queue is
        # in-order, so this trades ~1us of best-case start time for never
        # inserting an idle gap (which can reset the clock ramp) and for
        # running every REAL matmul at full clock.
        warm_w = consts.tile([P, P], F16, name="warm_w")
        nc.vector.memset(warm_w, 1.0)
        for _ in range(38):
            pw = psC.tile([P, LT], F32, tag="pc", name="pw")
            nc.tensor.matmul(pw[:32, :P], warm_w[:, :32], warm_w[:, :])

        # Resident weights, [p, kt, o] with the contraction 128-block on
        # partitions.  Step 0's stage A consumes them k-chunk by k-chunk, so
        # the DMAs are issued per 128-row chunk interleaved with step-0 q/v;
        # the fp8 chunks go first (they gate the first matmuls).
        w18s = consts.tile([P, NA8, H], F8E4)
        w28s = consts.tile([P, NA8, H], F8E4)
        q8s0 = qpool.tile([P, NA8, LT], F8E4, tag="q8s")
        v8s0 = qpool.tile([P, NA8, LT], F8E4, tag="v8s")
        # First four chunks issued from TWO queues in parallel (each DMA
        # issue occupies its queue ~600ns; the scalar queue is idle until
        # the first tanh) so the first matmuls gate ~1.2us earlier.
        nc.sync.dma_start(w18s, w18.rearrange("(nk p) h -> p nk h", p=P))
        nc.scalar.dma_start(w28s, w28.rearrange("(nk p) h -> p nk h", p=P))
        nc.sync.dma_start(q8s0, qT8[0, :, 0:LT].rearrange("(nk p) l -> p nk l", p=P))
        nc.scalar.dma_start(v8s0, vT8[0, :, 0:LT].rearrange("(nk p) l -> p nk l", p=P))
        w1s = consts.tile([P, NHF, H], F16)
        w2s = consts.tile([P, NHF, H], F16)
        qs0 = qpool.tile([P, NHF, LT], F16, tag="qs")
        vs0 = qpool.tile([P, NHF, LT], F16, tag="vs")
        for ht in range(NHF):
            rsl = slice(ht * P, (ht + 1) * P)
            nc.sync.dma_start(w1s[:, ht, :], w1t[rsl, :])
            nc.sync.dma_start(qs0[:, ht, :], qT[0, rsl, 0:LT])
            nc.sync.dma_start(w2s[:, ht, :], w2t[rsl, :])
            nc.sync.dma_start(vs0[:, ht, :], vT[0, rsl, 0:LT])
        b12s = consts.tile([P, NH], F32)
        nc.sync.dma_start(b12s, b12)
        vbs = consts.tile([P, NH], F32)
        nc.sync.dma_start(vbs, vbt)
        # Stage-B weights, queued right behind the startup chunks.  The DMA
        # queue is a SERIAL resource (each transfer occupies it for
        # ~bytes*3ns), so these are per-k-chunk: one coarse transfer parked
        # in front of a later-needed piece stalls the PE on that piece.
        vws = consts.tile([P, NH, H], F16)
        for ht in range(NH):
            nc.sync.dma_start(vws[:, ht, :], vwt[ht * P:(ht + 1) * P, :])

        steps = [(b, l0) for b in range(BLOC) for l0 in (0, LT)]

        vnat_tiles = {}

        def emit_stage_a_step0(hT):
            """k-chunk-OUTER stage A for step 0: 8 concurrent PSUM groups
            (all 8 banks), so each chunk iteration gates on just one k-chunk
            of w1/w2/q/v and compute starts ~5us earlier.  The fp8-DR chunks
            run first (smallest startup DMA)."""
            groups = []
            for gi, (pool, tg) in enumerate([(psA, "pa"), (psA, "pa"),
                                             (psB, "pb"), (psB, "pb"),
                                             (psC, "pc"), (psC, "pc"),
                                             (psC, "pc"), (psC, "pc")]):
                groups.append(pool.tile([P, LT], F32, tag=tg, name=f"g{gi}"))
            for o in range(NH):
                osl = slice(o * P, (o + 1) * P)
                nc.tensor.matmul(groups[o], w18s[:, :, osl], q8s0,
                                 start=True, stop=False, perf_mode=DR)
            for o in range(NH):
                osl = slice(o * P, (o + 1) * P)
                nc.tensor.matmul(groups[o], w28s[:, :, osl], v8s0,
                                 start=False, stop=False, perf_mode=DR)
            NSTREAM = 2  # last k-blocks run o-outer so the tanhs stream
            for ht in range(NHF - NSTREAM):
                for o in range(NH):
                    osl = slice(o * P, (o + 1) * P)
                    nc.tensor.matmul(groups[o], w1s[:, ht, osl], qs0[:, ht, :],
                                     start=False, stop=False)
                for o in range(NH):
                    osl = slice(o * P, (o + 1) * P)
                    nc.tensor.matmul(groups[o], w2s[:, ht, osl],
                                     vs0[:, ht, :], start=False, stop=False)
            for o in range(NH):
                osl = slice(o * P, (o + 1) * P)
                for ht in range(NHF - NSTREAM, NHF):
                    nc.tensor.matmul(groups[o], w1s[:, ht, osl], qs0[:, ht, :],
                                     start=False, stop=False)
                    nc.tensor.matmul(groups[o], w2s[:, ht, osl], vs0[:, ht, :],
                                     start=False, stop=(ht == NHF - 1))
                nc.scalar.activation(hT[:, o, :], groups[o], AFT.Tanh,
                                     bias=b12s[:, o:o + 1], scale=1.0)

        def emit_stage_a(i, b, l0):
            lsl = slice(l0, l0 + LT)
            hT = hpool.tile([P, NH, LT], F16, tag="hT")
            if i == 0:
                emit_stage_a_step0(hT)
            else:
                q8s = qpool.tile([P, NA8, LT], F8E4, tag="q8s")
                v8s = qpool.tile([P, NA8, LT], F8E4, tag="v8s")
                qs = qpool.tile([P, NHF, LT], F16, tag="qs")
                vs = qpool.tile([P, NHF, LT], F16, tag="vs")
                nc.sync.dma_start(
                    q8s, qT8[b, :, lsl].rearrange("(nk p) l -> p nk l", p=P))
                nc.sync.dma_start(
                    v8s, vT8[b, :, lsl].rearrange("(nk p) l -> p nk l", p=P))
                nc.sync.dma_start(
                    qs, qT[b, :, lsl].rearrange("(nh p) l -> p nh l", p=P))
                nc.sync.dma_start(
                    vs, vT[b, :, lsl].rearrange("(nh p) l -> p nh l", p=P))
                # Stage A: hT[o, l] = tanh(W1 q^T + W2 v^T + b1 + b2);
                # k-blocks 0-1 as one fp8-DR matmul per operand, rest f16
                for o in range(NH):
                    osl = slice(o * P, (o + 1) * P)
                    pa = psA.tile([P, LT], F32, tag="pa")
                    nc.tensor.matmul(pa, w18s[:, :, osl], q8s,
                                     start=True, stop=False, perf_mode=DR)
                    nc.tensor.matmul(pa, w28s[:, :, osl], v8s,
                                     start=False, stop=False, perf_mode=DR)
                    for ht in range(NHF):
                        nc.tensor.matmul(pa, w1s[:, ht, osl], qs[:, ht, :],
                                         start=False, stop=False)
                        nc.tensor.matmul(pa, w2s[:, ht, osl], vs[:, ht, :],
                                         start=False, stop=(ht == NHF - 1))
                    nc.scalar.activation(hT[:, o, :], pa, AFT.Tanh,
                                         bias=b12s[:, o:o + 1], scale=1.0)

            # value in fp8 natural [k, h] layout for the context matmul (used
            # ~a full step later).  Chunked so one coarse transfer doesn't
            # park in front of later-queued, sooner-needed pieces.
            if b not in vnat_tiles:
                vnat = vpool.tile([P, NH, H], F8E4, tag="vnat")
                for j in range(0, NH, 2):
                    nc.sync.dma_start(
                        vnat[:, j:j + 2, :],
                        vn[b, j * P:(j + 2) * P, :]
                        .rearrange("(nk p) h -> p nk h", p=P))
                vnat_tiles.clear()
                vnat_tiles[b] = vnat
            return hT

        def emit_stage_b(b, l0, hT, c0=0, cw=LT):
            """expT[o, l] = exp(Vw h + vb) in f16 (att numerator output) and
            centered fp8 expm1 for the context matmul.  (c0, cw) select a
            column sub-chunk of the l-tile (the last step runs two 256-wide
            halves so its un-hidden exp->sub->context tail is half as long)."""
            csl = slice(c0, c0 + cw)
            expT = epool.tile([P, NH, cw], F16, tag="expT")
            exc8 = epool.tile([P, NH, cw], F8E4, tag="exc8")
            for o in range(NH):
                osl = slice(o * P, (o + 1) * P)
                pb = psB.tile([P, cw], F32, tag="pb")
                for ht in range(NH):
                    nc.tensor.matmul(pb, vws[:, ht, osl], hT[:, ht, csl],
                                     start=(ht == 0), stop=(ht == NH - 1))
                nc.scalar.activation(expT[:, o, :], pb, AFT.Exp,
                                     bias=vbs[:, o:o + 1], scale=1.0)
                nc.vector.tensor_scalar_sub(exc8[:, o, :], expT[:, o, :], 1.0)
            nc.sync.dma_start(
                attT[b, :, l0 + c0:l0 + c0 + cw]
                .rearrange("(nh p) l -> p nh l", p=P), expT)
            return expT, exc8

        def emit_context(state, last=False):
            b, l0, c0, cw, exc8, vnat = state
            nlb = cw // P
            cs = cpool.tile([P, nlb, H], F16, tag="cs")
            # ctx_raw[l, h] = sum_k expm1_8[k, l] * v8[k, h], fp8 DoubleRow:
            # lhsT/rhs [p, 2, *] slices pair contraction rows (2t*128+p,
            # (2t+1)*128+p) on both sides.
            for lb in range(nlb):
                for hti in range(NHT):
                    hsl = slice(hti * LT, (hti + 1) * LT)
                    pc = psC.tile([P, LT], F32, tag="pc")
                    for t in range(0, NH, 2):
                        nc.tensor.matmul(pc,
                                         exc8[:, t:t + 2, lb * P:(lb + 1) * P],
                                         vnat[:, t:t + 2, hsl],
                                         start=(t == 0), stop=(t == NH - 2),
                                         perf_mode=DR)
                    # PSUM->SBUF evacuation alternating ScalarE/DVE so
                    # neither queue's backlog blocks psC slot reuse long
                    if hti == 0:
                        nc.scalar.activation(cs[:, lb, hsl], pc, AFT.Copy)
                    else:
                        nc.vector.tensor_copy(cs[:, lb, hsl], pc)
                if last:
                    # drain each row-block as soon as it lands, in 512-col
                    # halves, so the final evac+DMA chain is short
                    rsl = slice(l0 + c0 + lb * P, l0 + c0 + (lb + 1) * P)
                    nc.sync.dma_start(ctxo[b, rsl, 0:LT], cs[:, lb, 0:LT])
                    nc.sync.dma_start(ctxo[b, rsl, LT:H], cs[:, lb, LT:H])
            if not last:
                lsl = slice(l0 + c0, l0 + c0 + cw)
                nc.sync.dma_start(
                    ctxo[b, lsl, :].rearrange("(lb p) h -> p lb h", p=P),
                    cs[:, :nlb, :])

        pending = None
        for i, (b, l0) in enumerate(steps):
            hT = emit_stage_a(i, b, l0)
            if pending is not None:
                emit_context(pending)
            if i < len(steps) - 1:
                _, exc8 = emit_stage_b(b, l0, hT)
                pending = (b, l0, 0, LT, exc8, vnat_tiles[b])
            else:
                hw = LT // 2
                _, exc8a = emit_stage_b(b, l0, hT, 0, hw)
                _, exc8b = emit_stage_b(b, l0, hT, hw, hw)
                emit_context((b, l0, 0, hw, exc8a, vnat_tiles[b]), last=True)
                emit_context((b, l0, hw, hw, exc8b, vnat_tiles[b]), last=True)
                pending = None


def _get_program():
    if "nc" not in _PROGRAM_CACHE:
        _PROGRAM_CACHE["nc"] = _build_program()
    return _PROGRAM_CACHE["nc"]


def _prep_in_maps(query, value, w1_w, w1_b, w2_w, w2_b, v_w, v_b):
    f16 = np.float16
    f8 = ml_dtypes.float8_e4m3fn
    K8 = NA8 * P
    w1t = w1_w.T                        # [h, o] fp32
    w2t = w2_w.T
    vwt = v_w.T.astype(f16)
    b12 = np.ascontiguousarray((w1_b + w2_b).astype(np.float32).reshape(NH, P).T)
    vbt = np.ascontiguousarray(v_b.astype(np.float32).reshape(NH, P).T)
    w18 = np.ascontiguousarray(w1t[:K8] * WS).astype(f8)
    w28 = np.ascontiguousarray(w2t[:K8] * WS).astype(f8)
    w1tf = np.ascontiguousarray(w1t[K8:]).astype(f16)
    w2tf = np.ascontiguousarray(w2t[K8:]).astype(f16)

    in_maps = []
    for c in range(NCORES):
        sl = slice(c * BLOC, (c + 1) * BLOC)
        qt = query[sl].transpose(0, 2, 1)   # [b, h, l] fp32
        vt = value[sl].transpose(0, 2, 1)
        in_maps.append({
            "qt_in": np.ascontiguousarray(qt[:, K8:]).astype(f16),
            "vt_in": np.ascontiguousarray(vt[:, K8:]).astype(f16),
            "qt8_in": np.ascontiguousarray(qt[:, :K8] * (1.0 / WS)).astype(f8),
            "vt8_in": np.ascontiguousarray(vt[:, :K8] * (1.0 / WS)).astype(f8),
            "vn_in": value[sl].astype(f8),
            "w1t_in": w1tf,
            "w2t_in": w2tf,
            "w18_in": w18,
            "w28_in": w28,
            "vwt_in": vwt,
            "b12_in": b12,
            "vbt_in": vbt,
        })
    return in_maps


def run_sharded(inputs, **run_kwargs):
    """Build in_maps, run on 8 cores, return (att, ctx, BassKernelResults)."""
    query = np.asarray(inputs["query"], dtype=np.float32)
    value = np.asarray(inputs["value"], dtype=np.float32)
    in_maps = _prep_in_maps(
        query, value,
        np.asarray(inputs["w1_w"], np.float32), np.asarray(inputs["w1_b"], np.float32),
        np.asarray(inputs["w2_w"], np.float32), np.asarray(inputs["w2_b"], np.float32),
        np.asarray(inputs["v_w"], np.float32), np.asarray(inputs["v_b"], np.float32),
    )
    nc = _get_program()
    res = bass_utils.run_bass_kernel_spmd(
        nc, in_maps, core_ids=list(range(NCORES)), **run_kwargs)

    # Host-side softmax normalization + rank-1 de-centering (see docstring).
    att = np.empty((B, L, H), np.float32)
    ctxv = np.empty((B, L, H), np.float32)
    for c in range(NCORES):
        sl = slice(c * BLOC, (c + 1) * BLOC)
        att_num = res.results[c]["att_out"].transpose(0, 2, 1).astype(np.float32)
        ctx_raw = res.results[c]["ctx_out"].astype(np.float32)
        s = att_num.sum(axis=-1)[..., None]           # (BLOC, L, 1)
        colsum = value[sl].sum(axis=1)[:, None, :]    # (BLOC, 1, H)
        att[sl] = att_num / s
        ctxv[sl] = (ctx_raw + colsum) / s
    return att, ctxv, res


def kernel(**inputs):
    att, ctxv, _ = run_sharded(inputs)
    return att, ctxv


# revision 33
# speedup vs baseline: 1.0019x; 1.0019x over previous
"""Bahdanau attention kernel for Trainium2, 8-core data-parallel.

Problem (B=32, L=1024, H=1024, fp32):
    h     = tanh(q @ W1.T + b1 + v @ W2.T + b2)        # (B, L, H)
    score = h @ Vw.T + vb                              # (B, L, H)
    att   = softmax(score, axis=-1)                    # (B, L, H)
    ctx   = att @ v                                    # (B, L, H)  (bmm over kv dim)
    returns (att, ctx)

Strategy (v3):
  - Data-parallel: 4 batches per core on 8 cores.  Everything on-device runs
    in a TRANSPOSED layout [h, l] so the contraction dim always lands on SBUF
    partitions; host pre-transposes q/value and the weights.
  - The device computes only the three GEMM stages plus tanh/exp; the softmax
    NORMALIZATION runs on the host:  the device outputs the exp numerator
    (f16, also the att output pre-division) and an un-normalized context,
    and the host divides both by S = sum(exp).  This removes the on-device
    softmax-sum matmul, reciprocal, and att-mul, and breaks the
    recip -> mul -> context dependency chain at every step's tail.
  - Stage A/B matmuls run in fp16 (same PE speed as bf16, ~8x less
    quantization error), EXCEPT the first 2 of 8 contraction 128-blocks of
    both stage-A matmuls, which run as one fp8e4m3 DoubleRow matmul per
    operand (256-contraction at the same ~512 cycles, i.e. 2x).  More fp8
    blocks blow the att error budget (3 blocks -> 2.0e-2 in sim); fp8
    anywhere in stage B costs even more error per cycle saved.  The fp8
    pair is scaled as fp8(q/8) x fp8(8*w) -- the scale cancels in the
    product; it keeps the 0.02-std weights out of e4m3's subnormal range
    (which would cost ~15% element error) without pushing q too far in.
  - Context matmul runs entirely in fp8 DoubleRow.  Plain fp8 exp would
    cost ~3.9e-2 rel err (over the 2e-2 budget), so the operand is
    CENTERED:  exp(s) = 1 + expm1(s), where expm1(s) has std ~0.39 vs
    exp's mean ~1.07, so quantizing expm1 into fp8 carries 2.4x less
    absolute error.  The device computes ctx_raw = expm1_8 @ v8 and the
    host adds back the exact rank-1 term 1 @ v = colsum(v) before
    dividing by S.  Total measured on HW: att 1.67e-2, ctx 1.72e-2
    (CPU sim with ml_dtypes predicts HW error to ~2%: sim 1.69e-2).
  - expm1_8 is produced by a DVE tensor_scalar_sub (expT - 1) since the
    scalar engine has no Expm1 and cannot post-subtract; the DVE is
    otherwise nearly idle.
  - Per step (half-batch, 512 l-columns) the PE runs A (128 mm), then the
    PREVIOUS step's context (32 DR mm), then B (64 mm); the gap between
    B(i) and ctx(i) hides the scalar exp + DVE sub latency.
  - Step 0's stage A runs k-block-OUTER with 8 concurrent PSUM accumulation
    groups (all 8 banks), so the first matmul gates on one k-chunk of
    weights+inputs instead of ~3 MiB, and the DMA stream stays ahead of
    the PE for the rest of startup.  PE prewarm matmuls burn the startup
    DMA window so the HAM clock gate (1.2 -> 2.4 GHz after ~3.4us of PE
    activity) releases before the real matmuls start.
  - PSUM accumulation chains are kept short (<= 16); a 512-deep chain was
    observed to hard-crash the exec unit (NRT_EXEC_UNIT_UNRECOVERABLE).
"""

import numpy as np
import ml_dtypes
from contextlib import ExitStack

import concourse.bass as bass
import concourse.mybir as mybir
import concourse.tile as tile
from concourse import bacc, bass_utils

B, L, H = 32, 1024, 1024
NCORES = 8
BLOC = B // NCORES  # batches per core
P = 128             # partitions
LT = 512            # l-tile (moving free dim)
NLB = LT // P       # 128-row blocks per l-tile
NH = H // P         # 128-blocks along h / o / k
NHT = H // LT       # 512-tiles along h (context output)

NA8 = 2             # stage-A contraction 128-blocks (of 8) run in fp8-DR
NHF = NH - NA8      # remaining f16 128-blocks
WS = 8.0            # fp8 pair scale: stores fp8(q/WS) x fp8(WS*w) = q*w;
                    # keeps the 0.02-std weights out of e4m3's subnormal
                    # range without pushing q too far into it

F16 = mybir.dt.float16
F32 = mybir.dt.float32
F8E4 = mybir.dt.float8e4
AFT = mybir.ActivationFunctionType
DR = mybir.MatmulPerfMode.DoubleRow

_PROGRAM_CACHE = {}


def _build_program():
    nc = bacc.Bacc("TRN2", target_bir_lowering=False, debug=False)

    KF = NHF * P  # f16 contraction rows in stage A
    K8 = NA8 * P  # fp8 contraction rows
    qT = nc.dram_tensor("qt_in", [BLOC, KF, L], F16, kind="ExternalInput").ap()
    vT = nc.dram_tensor("vt_in", [BLOC, KF, L], F16, kind="ExternalInput").ap()
    qT8 = nc.dram_tensor("qt8_in", [BLOC, K8, L], F8E4, kind="ExternalInput").ap()
    vT8 = nc.dram_tensor("vt8_in", [BLOC, K8, L], F8E4, kind="ExternalInput").ap()
    vn = nc.dram_tensor("vn_in", [BLOC, L, H], F8E4, kind="ExternalInput").ap()
    w1t = nc.dram_tensor("w1t_in", [KF, H], F16, kind="ExternalInput").ap()
    w2t = nc.dram_tensor("w2t_in", [KF, H], F16, kind="ExternalInput").ap()
    w18 = nc.dram_tensor("w18_in", [K8, H], F8E4, kind="ExternalInput").ap()
    w28 = nc.dram_tensor("w28_in", [K8, H], F8E4, kind="ExternalInput").ap()
    vwt = nc.dram_tensor("vwt_in", [H, H], F16, kind="ExternalInput").ap()
    b12 = nc.dram_tensor("b12_in", [P, NH], F32, kind="ExternalInput").ap()
    vbt = nc.dram_tensor("vbt_in", [P, NH], F32, kind="ExternalInput").ap()

    attT = nc.dram_tensor("att_out", [BLOC, H, L], F16, kind="ExternalOutput").ap()
    ctxo = nc.dram_tensor("ctx_out", [BLOC, L, H], F16, kind="ExternalOutput").ap()

    with tile.TileContext(nc) as tc:
        _kernel_body(tc, qT, vT, qT8, vT8, vn, w1t, w2t, w18, w28, vwt,
                     b12, vbt, attT, ctxo)
    nc.compile()
    return nc


def _kernel_body(tc, qT, vT, qT8, vT8, vn, w1t, w2t, w18, w28, vwt,
                 b12, vbt, attT, ctxo):
    nc = tc.nc
    with ExitStack() as ctx:
        consts = ctx.enter_context(tc.tile_pool(name="consts", bufs=1))
        qpool = ctx.enter_context(tc.tile_pool(name="qpool", bufs=2))
        hpool = ctx.enter_context(tc.tile_pool(name="hpool", bufs=2))
        epool = ctx.enter_context(tc.tile_pool(name="epool", bufs=2))
        vpool = ctx.enter_context(tc.tile_pool(name="vpool", bufs=2))
        cpool = ctx.enter_context(tc.tile_pool(name="cpool", bufs=2))
        psA = ctx.enter_context(tc.tile_pool(name="psA", bufs=2, space="PSUM"))
        psB = ctx.enter_context(tc.tile_pool(name="psB", bufs=2, space="PSUM"))
        psC = ctx.enter_context(tc.tile_pool(name="psC", bufs=4, space="PSUM"))

        # PE prewarm (see module docstring).  The memset must NOT run on
        # GpSimd: its cold-start latency (~9us observed) would gate the
        # prewarm matmuls past the whole startup window.  40 matmuls
        # (~4.3us) deliberately overshoot the first operand chunks' arrival
        # (~9-11.5us wall, ~2us run-to-run variance): the PE queue is
        # in-order, so this trades ~1us of best-case start time for never
        # inserting an idle gap (which can reset the clock ramp) and for
        # running every REAL matmul at full clock.
        warm_w = consts.tile([P, P], F16, name="warm_w")
        nc.vector.memset(warm_w, 1.0)
        for _ in range(36):
            pw = psC.tile([P, LT], F32, tag="pc", name="pw")
            nc.tensor.matmul(pw[:32, :P], warm_w[:, :32], warm_w[:, :])

        # Resident weights, [p, kt, o] with the contraction 128-block on
        # partitions.  Step 0's stage A consumes them k-chunk by k-chunk, so
        # the DMAs are issued per 128-row chunk interleaved with step-0 q/v;
        # the fp8 chunks go first (they gate the first matmuls).
        w18s = consts.tile([P, NA8, H], F8E4)
        w28s = consts.tile([P, NA8, H], F8E4)
        q8s0 = qpool.tile([P, NA8, LT], F8E4, tag="q8s")
        v8s0 = qpool.tile([P, NA8, LT], F8E4, tag="v8s")
        # First four chunks issued from TWO queues in parallel (each DMA
        # issue occupies its queue ~600ns; the scalar queue is idle until
        # the first tanh) so the first matmuls gate ~1.2us earlier.
        nc.sync.dma_start(w18s, w18.rearrange("(nk p) h -> p nk h", p=P))
        nc.scalar.dma_start(w28s, w28.rearrange("(nk p) h -> p nk h", p=P))
        nc.sync.dma_start(q8s0, qT8[0, :, 0:LT].rearrange("(nk p) l -> p nk l", p=P))
        nc.scalar.dma_start(v8s0, vT8[0, :, 0:LT].rearrange("(nk p) l -> p nk l", p=P))
        w1s = consts.tile([P, NHF, H], F16)
        w2s = consts.tile([P, NHF, H], F16)
        qs0 = qpool.tile([P, NHF, LT], F16, tag="qs")
        vs0 = qpool.tile([P, NHF, LT], F16, tag="vs")
        for ht in range(NHF):
            rsl = slice(ht * P, (ht + 1) * P)
            nc.sync.dma_start(w1s[:, ht, :], w1t[rsl, :])
            nc.sync.dma_start(qs0[:, ht, :], qT[0, rsl, 0:LT])
            nc.sync.dma_start(w2s[:, ht, :], w2t[rsl, :])
            nc.sync.dma_start(vs0[:, ht, :], vT[0, rsl, 0:LT])
        b12s = consts.tile([P, NH], F32)
        nc.sync.dma_start(b12s, b12)
        vbs = consts.tile([P, NH], F32)
        nc.sync.dma_start(vbs, vbt)
        # Stage-B weights, queued right behind the startup chunks.  The DMA
        # queue is a SERIAL resource (each transfer occupies it for
        # ~bytes*3ns), so these are per-k-chunk: one coarse transfer parked
        # in front of a later-needed piece stalls the PE on that piece.
        vws = consts.tile([P, NH, H], F16)
        for ht in range(NH):
            nc.sync.dma_start(vws[:, ht, :], vwt[ht * P:(ht + 1) * P, :])

        steps = [(b, l0) for b in range(BLOC) for l0 in (0, LT)]

        vnat_tiles = {}

        def emit_stage_a_step0(hT):
            """k-chunk-OUTER stage A for step 0: 8 concurrent PSUM groups
            (all 8 banks), so each chunk iteration gates on just one k-chunk
            of w1/w2/q/v and compute starts ~5us earlier.  The fp8-DR chunks
            run first (smallest startup DMA)."""
            groups = []
            for gi, (pool, tg) in enumerate([(psA, "pa"), (psA, "pa"),
                                             (psB, "pb"), (psB, "pb"),
                                             (psC, "pc"), (psC, "pc"),
                                             (psC, "pc"), (psC, "pc")]):
                groups.append(pool.tile([P, LT], F32, tag=tg, name=f"g{gi}"))
            for o in range(NH):
                osl = slice(o * P, (o + 1) * P)
                nc.tensor.matmul(groups[o], w18s[:, :, osl], q8s0,
                                 start=True, stop=False, perf_mode=DR)
            for o in range(NH):
                osl = slice(o * P, (o + 1) * P)
                nc.tensor.matmul(groups[o], w28s[:, :, osl], v8s0,
                                 start=False, stop=False, perf_mode=DR)
            NSTREAM = 2  # last k-blocks run o-outer so the tanhs stream
            for ht in range(NHF - NSTREAM):
                for o in range(NH):
                    osl = slice(o * P, (o + 1) * P)
                    nc.tensor.matmul(groups[o], w1s[:, ht, osl], qs0[:, ht, :],
                                     start=False, stop=False)
                for o in range(NH):
                    osl = slice(o * P, (o + 1) * P)
                    nc.tensor.matmul(groups[o], w2s[:, ht, osl],
                                     vs0[:, ht, :], start=False, stop=False)
            for o in range(NH):
                osl = slice(o * P, (o + 1) * P)
                for ht in range(NHF - NSTREAM, NHF):
                    nc.tensor.matmul(groups[o], w1s[:, ht, osl], qs0[:, ht, :],
                                     start=False, stop=False)
                    nc.tensor.matmul(groups[o], w2s[:, ht, osl], vs0[:, ht, :],
                                     start=False, stop=(ht == NHF - 1))
                nc.scalar.activation(hT[:, o, :], groups[o], AFT.Tanh,
                                     bias=b12s[:, o:o + 1], scale=1.0)

        def emit_stage_a(i, b, l0):
            lsl = slice(l0, l0 + LT)
            hT = hpool.tile([P, NH, LT], F16, tag="hT")
            if i == 0:
                emit_stage_a_step0(hT)
            else:
                q8s = qpool.tile([P, NA8, LT], F8E4, tag="q8s")
                v8s = qpool.tile([P, NA8, LT], F8E4, tag="v8s")
                qs = qpool.tile([P, NHF, LT], F16, tag="qs")
                vs = qpool.tile([P, NHF, LT], F16, tag="vs")
                nc.sync.dma_start(
                    q8s, qT8[b, :, lsl].rearrange("(nk p) l -> p nk l", p=P))
                nc.sync.dma_start(
                    v8s, vT8[b, :, lsl].rearrange("(nk p) l -> p nk l", p=P))
                nc.sync.dma_start(
                    qs, qT[b, :, lsl].rearrange("(nh p) l -> p nh l", p=P))
                nc.sync.dma_start(
                    vs, vT[b, :, lsl].rearrange("(nh p) l -> p nh l", p=P))
                # Stage A: hT[o, l] = tanh(W1 q^T + W2 v^T + b1 + b2);
                # k-blocks 0-1 as one fp8-DR matmul per operand, rest f16
                for o in range(NH):
                    osl = slice(o * P, (o + 1) * P)
                    pa = psA.tile([P, LT], F32, tag="pa")
                    nc.tensor.matmul(pa, w18s[:, :, osl], q8s,
                                     start=True, stop=False, perf_mode=DR)
                    nc.tensor.matmul(pa, w28s[:, :, osl], v8s,
                                     start=False, stop=False, perf_mode=DR)
                    for ht in range(NHF):
                        nc.tensor.matmul(pa, w1s[:, ht, osl], qs[:, ht, :],
                                         start=False, stop=False)
                        nc.tensor.matmul(pa, w2s[:, ht, osl], vs[:, ht, :],
                                         start=False, stop=(ht == NHF - 1))
                    nc.scalar.activation(hT[:, o, :], pa, AFT.Tanh,
                                         bias=b12s[:, o:o + 1], scale=1.0)

            # value in fp8 natural [k, h] layout for the context matmul (used
            # ~a full step later).  Chunked so one coarse transfer doesn't
            # park in front of later-queued, sooner-needed pieces.
            if b not in vnat_tiles:
                vnat = vpool.tile([P, NH, H], F8E4, tag="vnat")
                for j in range(0, NH, 2):
                    nc.sync.dma_start(
                        vnat[:, j:j + 2, :],
                        vn[b, j * P:(j + 2) * P, :]
                        .rearrange("(nk p) h -> p nk h", p=P))
                vnat_tiles.clear()
                vnat_tiles[b] = vnat
            return hT

        def emit_stage_b(b, l0, hT, c0=0, cw=LT):
            """expT[o, l] = exp(Vw h + vb) in f16 (att numerator output) and
            centered fp8 expm1 for the context matmul.  (c0, cw) select a
            column sub-chunk of the l-tile (the last step runs two 256-wide
            halves so its un-hidden exp->sub->context tail is half as long)."""
            csl = slice(c0, c0 + cw)
            expT = epool.tile([P, NH, cw], F16, tag="expT")
            exc8 = epool.tile([P, NH, cw], F8E4, tag="exc8")
            for o in range(NH):
                osl = slice(o * P, (o + 1) * P)
                pb = psB.tile([P, cw], F32, tag="pb")
                for ht in range(NH):
                    nc.tensor.matmul(pb, vws[:, ht, osl], hT[:, ht, csl],
                                     start=(ht == 0), stop=(ht == NH - 1))
                nc.scalar.activation(expT[:, o, :], pb, AFT.Exp,
                                     bias=vbs[:, o:o + 1], scale=1.0)
                nc.vector.tensor_scalar_sub(exc8[:, o, :], expT[:, o, :], 1.0)
            nc.sync.dma_start(
                attT[b, :, l0 + c0:l0 + c0 + cw]
                .rearrange("(nh p) l -> p nh l", p=P), expT)
            return expT, exc8

        def emit_context(state, last=False):
            b, l0, c0, cw, exc8, vnat = state
            nlb = cw // P
            cs = cpool.tile([P, nlb, H], F16, tag="cs")
            # ctx_raw[l, h] = sum_k expm1_8[k, l] * v8[k, h], fp8 DoubleRow:
            # lhsT/rhs [p, 2, *] slices pair contraction rows (2t*128+p,
            # (2t+1)*128+p) on both sides.
            for lb in range(nlb):
                for hti in range(NHT):
                    hsl = slice(hti * LT, (hti + 1) * LT)
                    pc = psC.tile([P, LT], F32, tag="pc")
                    for t in range(0, NH, 2):
                        nc.tensor.matmul(pc,
                                         exc8[:, t:t + 2, lb * P:(lb + 1) * P],
                                         vnat[:, t:t + 2, hsl],
                                         start=(t == 0), stop=(t == NH - 2),
                                         perf_mode=DR)
                    # PSUM->SBUF evacuation alternating ScalarE/DVE so
                    # neither queue's backlog blocks psC slot reuse long
                    if hti == 0:
                        nc.scalar.activation(cs[:, lb, hsl], pc, AFT.Copy)
                    else:
                        nc.vector.tensor_copy(cs[:, lb, hsl], pc)
                if last:
                    # drain each row-block as soon as it lands, in 512-col
                    # halves, so the final evac+DMA chain is short
                    rsl = slice(l0 + c0 + lb * P, l0 + c0 + (lb + 1) * P)
                    nc.sync.dma_start(ctxo[b, rsl, 0:LT], cs[:, lb, 0:LT])
                    nc.sync.dma_start(ctxo[b, rsl, LT:H], cs[:, lb, LT:H])
            if not last:
                lsl = slice(l0 + c0, l0 + c0 + cw)
                nc.sync.dma_start(
                    ctxo[b, lsl, :].rearrange("(lb p) h -> p lb h", p=P),
                    cs[:, :nlb, :])

        pending = None
        for i, (b, l0) in enumerate(steps):
            hT = emit_stage_a(i, b, l0)
            if pending is not None:
                emit_context(pending)
            if i < len(steps) - 1:
                _, exc8 = emit_stage_b(b, l0, hT)
                pending = (b, l0, 0, LT, exc8, vnat_tiles[b])
            else:
                hw = LT // 2
                _, exc8a = emit_stage_b(b, l0, hT, 0, hw)
                _, exc8b = emit_stage_b(b, l0, hT, hw, hw)
                emit_context((b, l0, 0, hw, exc8a, vnat_tiles[b]), last=True)
                emit_context((b, l0, hw, hw, exc8b, vnat_tiles[b]), last=True)
                pending = None


def _get_program():
    if "nc" not in _PROGRAM_CACHE:
        _PROGRAM_CACHE["nc"] = _build_program()
    return _PROGRAM_CACHE["nc"]


def _prep_in_maps(query, value, w1_w, w1_b, w2_w, w2_b, v_w, v_b):
    f16 = np.float16
    f8 = ml_dtypes.float8_e4m3fn
    K8 = NA8 * P
    w1t = w1_w.T                        # [h, o] fp32
    w2t = w2_w.T
    vwt = v_w.T.astype(f16)
    b12 = np.ascontiguousarray((w1_b + w2_b).astype(np.float32).reshape(NH, P).T)
    vbt = np.ascontiguousarray(v_b.astype(np.float32).reshape(NH, P).T)
    w18 = np.ascontiguousarray(w1t[:K8] * WS).astype(f8)
    w28 = np.ascontiguousarray(w2t[:K8] * WS).astype(f8)
    w1tf = np.ascontiguousarray(w1t[K8:]).astype(f16)
    w2tf = np.ascontiguousarray(w2t[K8:]).astype(f16)

    in_maps = []
    for c in range(NCORES):
        sl = slice(c * BLOC, (c + 1) * BLOC)
        qt = query[sl].transpose(0, 2, 1)   # [b, h, l] fp32
        vt = value[sl].transpose(0, 2, 1)
        in_maps.append({
            "qt_in": np.ascontiguousarray(qt[:, K8:]).astype(f16),
            "vt_in": np.ascontiguousarray(vt[:, K8:]).astype(f16),
            "qt8_in": np.ascontiguousarray(qt[:, :K8] * (1.0 / WS)).astype(f8),
            "vt8_in": np.ascontiguousarray(vt[:, :K8] * (1.0 / WS)).astype(f8),
            "vn_in": value[sl].astype(f8),
            "w1t_in": w1tf,
            "w2t_in": w2tf,
            "w18_in": w18,
            "w28_in": w28,
            "vwt_in": vwt,
            "b12_in": b12,
            "vbt_in": vbt,
        })
    return in_maps


def run_sharded(inputs, **run_kwargs):
    """Build in_maps, run on 8 cores, return (att, ctx, BassKernelResults)."""
    query = np.asarray(inputs["query"], dtype=np.float32)
    value = np.asarray(inputs["value"], dtype=np.float32)
    in_maps = _prep_in_maps(
        query, value,
        np.asarray(inputs["w1_w"], np.float32), np.asarray(inputs["w1_b"], np.float32),
        np.asarray(inputs["w2_w"], np.float32), np.asarray(inputs["w2_b"], np.float32),
        np.asarray(inputs["v_w"], np.float32), np.asarray(inputs["v_b"], np.float32),
    )
    nc = _get_program()
    res = bass_utils.run_bass_kernel_spmd(
        nc, in_maps, core_ids=list(range(NCORES)), **run_kwargs)

    # Host-side softmax normalization + rank-1 de-centering (see docstring).
    att = np.empty((B, L, H), np.float32)
    ctxv = np.empty((B, L, H), np.float32)
    for c in range(NCORES):
        sl = slice(c * BLOC, (c + 1) * BLOC)
        att_num = res.results[c]["att_out"].transpose(0, 2, 1).astype(np.float32)
        ctx_raw = res.results[c]["ctx_out"].astype(np.float32)
        s = att_num.sum(axis=-1)[..., None]           # (BLOC, L, 1)
        colsum = value[sl].sum(axis=1)[:, None, :]    # (BLOC, 1, H)
        att[sl] = att_num / s
        ctxv[sl] = (ctx_raw + colsum) / s
    return att, ctxv, res


def kernel(**inputs):
    att, ctxv, _ = run_sharded(inputs)
    return att, ctxv
